# revision 16
# baseline (speedup 1.0000x reference)
"""Trainium2 Bass kernel for nn_Graph_to_Featuremaps_savemem.

Reference computation:
    scores[b,p,n] = s_res[b,p] + s_hid[b,n];  attn = softmax_n(scores)
    out[b,c,p]    = relu(sum_n attn[b,p,n] * (x[b,n,:] @ W)[c])

Key simplification: softmax over n is shift-invariant, so the per-pixel
s_res[b,p] term (the only use of res_feature / node_fea_for_res) cancels:
    attn[b,p,n] = softmax_n(s_hid[b,n])   (independent of p)
    out[b,c,p]  = relu(sum_n a[b,n] * nv[b,n,c])  broadcast over all pixels.

So the kernel is a tiny softmax-weighted matmul per batch followed by a
151 MB broadcast-write of the (B, C) result over H*W pixels. Sharding:
data-parallel over batch, 2 batches per core across 8 cores; the small
params (node_fea_for_hidden, weight) are replicated.

The structure targets the DMA-store roofline (~26 GB/s x 16 DMA engines
per core): the 18.9 MB/core output stream goes on the wire as early as
possible and everything else hides beneath it.

  - The output broadcast is NOT materialized in SBUF.  Per (batch, c-half)
    only one (128, CH) chunk is filled (CH = 2304 pixels); the store DMA's
    *source* access pattern revisits it with a stride-0 repeat dim, so the
    DMA replicates it across all 9216 pixels.  This removes the baseline's
    2x 9.4 MB DVE broadcast fills (23.8 us) from the critical path.
    CH is chosen so descriptors are 9.2 KB: at 4.6 KB the descriptor
    generator falls ~6% short of the 16-engine line rate and the last
    engine in the round-robin accumulates an 8 us straggle.
  - All DMAs ride the sync-engine queue (its trigger is ~2x faster than
    the scalar engine's, and queue FIFO order lets the tail drain wait on
    the final DMA's semaphore alone).
  - s_hid = x . nfh is a DVE multiply + free-dim reduce against a
    host-packed nfh replica -- no PE transposes anywhere.
  - softmax normalization is deferred: y = x^T (blockmask * exp(s)) and
    v = W^T y use unnormalized weights; 1/denom and the ReLU are fused
    into the chunk fills (DVE tensor_scalar mult+max for the low c-half,
    scalar-engine activation Relu-with-scale for the high c-half, running
    concurrently).  v and 1/denom are funneled through GPSIMD copies so
    every fill carries a single sync wait (HW limit).
  - matmuls run in bf16 (O(1) gaussian data; tolerance 2e-2, measured
    error ~3e-3).
"""

import numpy as np

import concourse.bass as bass
import concourse.mybir as mybir
import concourse.tile as tile
from concourse.bass_utils import run_bass_kernel_spmd

B, NODES, HID, C, H, W = 16, 7, 256, 256, 96, 96
P = H * W                # 9216 pixels
NCORES = 8
BL = B // NCORES         # 2 local batches per core
BN = BL * NODES          # 14 (b,n) rows
CH1 = 4608               # chunk width of the FIRST store DMA (pixels)
NREP1 = P // CH1         # its stride-0 repeat count

# cin_a (small, loaded first; only rows 32:46 are transferred):
#   cols 0:256 x[(b n), h]; 256:512 nfh replicated per row; 512:514 blockmask
XROW = 32                # base partition for the 14 (b,n) rows (PE: 0/32/64)
COL_X = 0
COL_NFH = 256
COL_BM = 512
CINA_COLS = 514
# cin_b: W packed [k, kh*256 + c] (k = h % 128, kh = h // 128)
CINB_COLS = 2 * C

_cache: dict = {}


def _rep_ap(ap, dims):
    """Return a copy of `ap` with its non-partition dims replaced by `dims`
    (list of [stride, count]); used to build stride-0 broadcast patterns."""
    a = ap.copy()
    a.ap = mybir.VecI64Pair([list(a.ap[0])] + [list(d) for d in dims])
    return a


def _build_nc():
    nc = bass.Bass()
    f32 = mybir.dt.float32
    bf16 = mybir.dt.bfloat16
    cina_d = nc.declare_dram_parameter("cina", [128, CINA_COLS], f32, isOutput=False)
    cinb_d = nc.declare_dram_parameter("cinb", [128, CINB_COLS], f32, isOutput=False)
    out_d = nc.declare_dram_parameter("out", [BL, C, P], f32, isOutput=True)

    with tile.TileContext(nc) as tc:
        with (
            tc.tile_pool(name="sb", bufs=1) as sb,
            tc.tile_pool(name="ps", bufs=1, space=bass.MemorySpace.PSUM) as ps,
        ):
            cina = sb.tile([128, CINA_COLS], f32)
            cinb = sb.tile([128, CINB_COLS], f32)
            # The two input loads trigger concurrently from different
            # engines, so the weight lands before the exp(s) chain needs the
            # DVE and its cast never blocks the critical path.
            nc.sync.dma_start(
                out=cina[XROW : XROW + BN, :], in_=cina_d[XROW : XROW + BN, :]
            )
            nc.scalar.dma_start(out=cinb[:], in_=cinb_d[:])

            x_sl = cina[XROW : XROW + BN, COL_X : COL_X + HID]
            nfh_sl = cina[XROW : XROW + BN, COL_NFH : COL_NFH + HID]
            bm_sl = cina[XROW : XROW + BN, COL_BM : COL_BM + BL]

            # DVE-produced matmul operands (single-producer rule for PE).
            ones_col = sb.tile([128, 1], bf16)
            nc.vector.memset(ones_col[:], 1.0)
            ones_row = sb.tile([1, 128], bf16)
            nc.vector.memset(ones_row[:], 1.0)

            # s[(b n)] = sum_h x * nfh  (multiply + free-dim reduce).
            tt_scratch = sb.tile([128, HID], f32)
            s_col = sb.tile([128, 1], f32)
            nc.vector.tensor_tensor(
                out=tt_scratch[XROW : XROW + BN, :],
                in0=x_sl,
                in1=nfh_sl,
                op=mybir.AluOpType.mult,
            )
            nc.vector.tensor_reduce(
                out=s_col[XROW : XROW + BN, :],
                in_=tt_scratch[XROW : XROW + BN, :],
                axis=mybir.AxisListType.X,
                op=mybir.AluOpType.add,
            )
            sb_x = sb.tile([128, HID], bf16)
            nc.vector.tensor_copy(out=sb_x[XROW : XROW + BN, :], in_=x_sl)

            # e = exp(s) on the scalar engine (normalization deferred).
            e_col = sb.tile([128, 1], f32)
            nc.scalar.activation(
                e_col[XROW : XROW + BN, :],
                s_col[XROW : XROW + BN, :],
                mybir.ActivationFunctionType.Exp,
            )
            # rhs_e[(b n), b'] = blockmask * e  (unnormalized per-batch attn).
            rhs_e = sb.tile([128, BL], bf16)
            nc.vector.tensor_scalar(
                out=rhs_e[XROW : XROW + BN, :],
                in0=bm_sl,
                scalar1=e_col[XROW : XROW + BN, 0:1],
                scalar2=None,
                op0=mybir.AluOpType.mult,
            )
            # Weight cast placed AFTER rhs_e in the DVE stream: it is 430 ns
            # of DVE time and must not delay the critical exp->rhs_e->y path
            # (the tile scheduler keeps per-engine program order here).
            sb_w = sb.tile([128, CINB_COLS], bf16)
            nc.vector.tensor_copy(out=sb_w[:], in_=cinb[:])

            # denom[b] = sum_n e ; y[h, b] = sum_n x * e  (contract over bn).
            ps_den = ps.tile([1, BL], f32, tag="den")
            nc.tensor.matmul(
                ps_den[:],
                ones_col[XROW : XROW + BN, :],
                rhs_e[XROW : XROW + BN, :],
                start=True,
                stop=True,
            )
            ps_y = ps.tile([128, 2 * BL], f32, tag="y")
            for kh in range(2):
                nc.tensor.matmul(
                    ps_y[:, kh * BL : (kh + 1) * BL],
                    sb_x[XROW : XROW + BN, kh * 128 : (kh + 1) * 128],
                    rhs_e[XROW : XROW + BN, :],
                    start=True,
                    stop=True,
                )
            recip = sb.tile([1, BL], bf16)
            with nc.allow_low_precision(reason="1/denom in bf16; tol 2e-2"):
                nc.vector.reciprocal(recip[:], ps_den[:])
            s_y = sb.tile([128, 2 * BL], bf16)
            nc.vector.tensor_copy(out=s_y[:], in_=ps_y[:])

            # v[c, b] = sum_h W[h, c] * y[h, b]   (c-half per group).
            ps_v = ps.tile([128, 2 * BL], f32, tag="v")
            for ch in range(2):
                for kh in range(2):
                    nc.tensor.matmul(
                        ps_v[:, ch * BL : (ch + 1) * BL],
                        sb_w[:, kh * C + ch * 128 : kh * C + (ch + 1) * 128],
                        s_y[:, kh * BL : (kh + 1) * BL],
                        start=(kh == 0),
                        stop=(kh == 1),
                    )

            # Broadcast 1/denom to all partitions with a K=1 matmul, placed
            # AFTER the v matmuls so its reciprocal wait never stalls them
            # (GPSIMD cannot read PSUM and DVE lanes cannot cross
            # partitions), then funnel v and 1/denom to SBUF on DVE so
            # every fill below needs at most one sync wait (HW limit).
            ps_r = ps.tile([128, BL], f32, tag="r")
            nc.tensor.matmul(ps_r[:], ones_row[:], recip[:], start=True, stop=True)
            s_v = sb.tile([128, 2 * BL], f32)
            nc.vector.tensor_copy(out=s_v[:], in_=ps_v[:])
            s_rr = sb.tile([128, BL], f32)
            nc.vector.tensor_copy(out=s_rr[:], in_=ps_r[:])

            # Normalize + ReLU + broadcast-fill chunks, then store.  The
            # FIRST DMA uses a narrow CH1-wide chunk (fast fill; its store
            # DMA replicates it over all pixels via a stride-0 repeat dim
            # in the source access pattern) so the stream starts early; the
            # other three use full 9216-wide chunks whose 36.9 KB
            # descriptors keep every DMA engine at line rate with zero
            # descriptor-generation ramp (narrower descriptors starve the
            # last round-robin engine at each DMA start, costing 1-2.5 us
            # per DMA on the stream tail).  Low c-halves fill on DVE, high
            # c-halves on the scalar engine (activation Relu with
            # per-partition scale); the two engines fill concurrently.
            bc0 = sb.tile([128, CH1 + P], f32, tag="bc0")
            bc1 = sb.tile([128, 2 * P], f32, tag="bc1")

            def dve_fill(dst, b, width):
                nc.vector.tensor_scalar(
                    out=dst,
                    in0=_rep_ap(s_v[:, b : b + 1], [[0, width]]),
                    scalar1=s_rr[:, b : b + 1],
                    scalar2=0.0,
                    op0=mybir.AluOpType.mult,
                    op1=mybir.AluOpType.max,
                )

            def act_fill(dst, b, width):
                nc.scalar.activation(
                    dst,
                    _rep_ap(s_v[:, BL + b : BL + b + 1], [[0, width]]),
                    mybir.ActivationFunctionType.Relu,
                    scale=s_rr[:, b : b + 1],
                )

            dve_fill(bc0[:, 0:CH1], 0, CH1)
            nc.sync.dma_start(
                out=_rep_ap(out_d[0][0:128, :], [[CH1, NREP1], [1, CH1]]),
                in_=_rep_ap(bc0[:, 0:CH1], [[0, NREP1], [1, CH1]]),
            )
            act_fill(bc0[:, CH1 : CH1 + P], 0, P)
            nc.sync.dma_start(out=out_d[0][128:256, :], in_=bc0[:, CH1 : CH1 + P])
            dve_fill(bc1[:, 0:P], 1, P)
            nc.sync.dma_start(out=out_d[1][0:128, :], in_=bc1[:, 0:P])
            act_fill(bc1[:, P : 2 * P], 1, P)
            nc.sync.dma_start(out=out_d[1][128:256, :], in_=bc1[:, P : 2 * P])
    _fix_tail_drain(nc)
    return nc


def _fix_tail_drain(nc):
    """Walrus accepts very few sync waits per instruction, and Tile's
    kernel-tail drain waits on every semaphore. The whole dataflow funnels
    into the four output DMAs, all FIFO on the sync queue, so waiting on
    the LAST one's completion sem alone is sufficient. Strip the drain
    down to that wait."""
    import bass_rust

    out_sem = None
    for ins in nc.inst_map.values():
        if type(ins).__name__ == "InstDMACopy" and "out_set" in str(ins):
            si = ins.sync_info
            if si is not None and len(si.on_update) > 0:
                out_sem = si.on_update[0].ant_name
    assert out_sem is not None, "output DMA completion sem not found"
    for ins in nc.inst_map.values():
        si = ins.sync_info
        if type(ins).__name__ == "InstDrain" and si is not None and len(si.on_wait) > 1:
            keep = [w for w in si.on_wait if w.ant_name == out_sem]
            assert len(keep) == 1, (out_sem, [w.ant_name for w in si.on_wait])
            ins.sync_info = bass_rust.SyncInfo(
                on_wait=keep, on_update=list(si.on_update)
            )


def _get_nc():
    if "nc" not in _cache:
        _cache["nc"] = _build_nc()
    return _cache["nc"]


def _pack_cina(x_shard, nfh):
    cina = np.zeros((128, CINA_COLS), dtype=np.float32)
    cina[XROW : XROW + BN, COL_X : COL_X + HID] = x_shard.reshape(BN, HID)
    cina[XROW : XROW + BN, COL_NFH : COL_NFH + HID] = nfh[:, 0][None, :]
    for b in range(BL):
        cina[XROW + b * NODES : XROW + (b + 1) * NODES, COL_BM + b] = 1.0
    return cina


def _pack_cinb(w):
    cinb = np.zeros((128, CINB_COLS), dtype=np.float32)
    cinb[:, 0:C] = w[0:128, :]
    cinb[:, C : 2 * C] = w[128:256, :]
    return cinb


def _make_in_maps(input, node_fea_for_hidden, weight):
    x_full = np.asarray(input, dtype=np.float32)[0]  # (B, N, HID)
    nfh = np.asarray(node_fea_for_hidden, dtype=np.float32)
    w = np.asarray(weight, dtype=np.float32)
    cinb = _pack_cinb(w)
    return [
        {"cina": _pack_cina(x_full[i * BL : (i + 1) * BL], nfh), "cinb": cinb}
        for i in range(NCORES)
    ]


def _run(in_maps, trace=False, **kwargs):
    nc = _get_nc()
    return run_bass_kernel_spmd(nc, in_maps, list(range(NCORES)), trace=trace, **kwargs)


def kernel(input, res_feature, node_fea_for_res, node_fea_for_hidden, weight):
    in_maps = _make_in_maps(input, node_fea_for_hidden, weight)
    res = _run(in_maps)
    shards = [res.results[i]["out"] for i in range(NCORES)]  # each (BL, C, P)
    full = np.concatenate(shards, axis=0)  # (B, C, P)
    return full.reshape(B, C, H, W).astype(np.float32, copy=False)


# revision 17
# speedup vs baseline: 1.0014x; 1.0014x over previous
"""Trainium2 Bass kernel for nn_Graph_to_Featuremaps_savemem.

Reference computation:
    scores[b,p,n] = s_res[b,p] + s_hid[b,n];  attn = softmax_n(scores)
    out[b,c,p]    = relu(sum_n attn[b,p,n] * (x[b,n,:] @ W)[c])

Key simplification: softmax over n is shift-invariant, so the per-pixel
s_res[b,p] term (the only use of res_feature / node_fea_for_res) cancels:
    attn[b,p,n] = softmax_n(s_hid[b,n])   (independent of p)
    out[b,c,p]  = relu(sum_n a[b,n] * nv[b,n,c])  broadcast over all pixels.

So the kernel is a tiny softmax-weighted matmul per batch followed by a
151 MB broadcast-write of the (B, C) result over H*W pixels. Sharding:
data-parallel over batch, 2 batches per core across 8 cores; the small
params (node_fea_for_hidden, weight) are replicated.

The structure targets the DMA-store roofline (~26 GB/s x 16 DMA engines
per core): the 18.9 MB/core output stream goes on the wire as early as
possible and everything else hides beneath it.

  - The output broadcast is NOT materialized in SBUF.  Per (batch, c-half)
    only one (128, CH) chunk is filled (CH = 2304 pixels); the store DMA's
    *source* access pattern revisits it with a stride-0 repeat dim, so the
    DMA replicates it across all 9216 pixels.  This removes the baseline's
    2x 9.4 MB DVE broadcast fills (23.8 us) from the critical path.
    CH is chosen so descriptors are 9.2 KB: at 4.6 KB the descriptor
    generator falls ~6% short of the 16-engine line rate and the last
    engine in the round-robin accumulates an 8 us straggle.
  - All DMAs ride the sync-engine queue (its trigger is ~2x faster than
    the scalar engine's, and queue FIFO order lets the tail drain wait on
    the final DMA's semaphore alone).
  - s_hid = x . nfh is a DVE multiply + free-dim reduce against a
    host-packed nfh replica -- no PE transposes anywhere.
  - softmax normalization is deferred: y = x^T (blockmask * exp(s)) and
    v = W^T y use unnormalized weights; 1/denom and the ReLU are fused
    into the chunk fills (DVE tensor_scalar mult+max for the low c-half,
    scalar-engine activation Relu-with-scale for the high c-half, running
    concurrently).  v and 1/denom are funneled through GPSIMD copies so
    every fill carries a single sync wait (HW limit).
  - matmuls run in bf16 (O(1) gaussian data; tolerance 2e-2, measured
    error ~3e-3).
"""

import numpy as np

import concourse.bass as bass
import concourse.mybir as mybir
import concourse.tile as tile
from concourse.bass_utils import run_bass_kernel_spmd

B, NODES, HID, C, H, W = 16, 7, 256, 256, 96, 96
P = H * W                # 9216 pixels
NCORES = 8
BL = B // NCORES         # 2 local batches per core
BN = BL * NODES          # 14 (b,n) rows
CH1 = 4608               # chunk width of the FIRST store DMA (pixels)
NREP1 = P // CH1         # its stride-0 repeat count

# cin_a (small, loaded first; only rows 32:46 are transferred):
#   cols 0:256 x[(b n), h]; 256:512 nfh replicated per row; 512:514 blockmask
XROW = 32                # base partition for the 14 (b,n) rows (PE: 0/32/64)
COL_X = 0
COL_NFH = 256
COL_BM = 512
CINA_COLS = 514
# cin_b: W packed [k, kh*256 + c] (k = h % 128, kh = h // 128)
CINB_COLS = 2 * C

_cache: dict = {}


def _rep_ap(ap, dims):
    """Return a copy of `ap` with its non-partition dims replaced by `dims`
    (list of [stride, count]); used to build stride-0 broadcast patterns."""
    a = ap.copy()
    a.ap = mybir.VecI64Pair([list(a.ap[0])] + [list(d) for d in dims])
    return a


def _build_nc():
    nc = bass.Bass()
    f32 = mybir.dt.float32
    bf16 = mybir.dt.bfloat16
    cina_d = nc.declare_dram_parameter("cina", [128, CINA_COLS], f32, isOutput=False)
    cinb_d = nc.declare_dram_parameter("cinb", [128, CINB_COLS], f32, isOutput=False)
    out_d = nc.declare_dram_parameter("out", [BL, C, P], f32, isOutput=True)

    with tile.TileContext(nc) as tc:
        with (
            tc.tile_pool(name="sb", bufs=1) as sb,
            tc.tile_pool(name="ps", bufs=1, space=bass.MemorySpace.PSUM) as ps,
        ):
            cina = sb.tile([128, CINA_COLS], f32)
            cinb = sb.tile([128, CINB_COLS], f32)
            # The two input loads trigger concurrently from different
            # engines, so the weight lands before the exp(s) chain needs the
            # DVE and its cast never blocks the critical path.
            nc.sync.dma_start(
                out=cina[XROW : XROW + BN, :], in_=cina_d[XROW : XROW + BN, :]
            )
            nc.scalar.dma_start(out=cinb[:], in_=cinb_d[:])

            x_sl = cina[XROW : XROW + BN, COL_X : COL_X + HID]
            nfh_sl = cina[XROW : XROW + BN, COL_NFH : COL_NFH + HID]
            bm_sl = cina[XROW : XROW + BN, COL_BM : COL_BM + BL]

            # DVE-produced matmul operands (single-producer rule for PE).
            ones_col = sb.tile([128, 1], bf16)
            nc.vector.memset(ones_col[:], 1.0)
            ones_row = sb.tile([1, 128], bf16)
            nc.vector.memset(ones_row[:], 1.0)

            # s[(b n)] = sum_h x * nfh  (multiply + free-dim reduce).
            tt_scratch = sb.tile([128, HID], f32)
            s_col = sb.tile([128, 1], f32)
            nc.vector.tensor_tensor(
                out=tt_scratch[XROW : XROW + BN, :],
                in0=x_sl,
                in1=nfh_sl,
                op=mybir.AluOpType.mult,
            )
            nc.vector.tensor_reduce(
                out=s_col[XROW : XROW + BN, :],
                in_=tt_scratch[XROW : XROW + BN, :],
                axis=mybir.AxisListType.X,
                op=mybir.AluOpType.add,
            )
            sb_x = sb.tile([128, HID], bf16)
            nc.vector.tensor_copy(out=sb_x[XROW : XROW + BN, :], in_=x_sl)

            # e = exp(s) on the scalar engine (normalization deferred).
            e_col = sb.tile([128, 1], f32)
            nc.scalar.activation(
                e_col[XROW : XROW + BN, :],
                s_col[XROW : XROW + BN, :],
                mybir.ActivationFunctionType.Exp,
            )
            # rhs_e[(b n), b'] = blockmask * e  (unnormalized per-batch attn).
            rhs_e = sb.tile([128, BL], bf16)
            nc.vector.tensor_scalar(
                out=rhs_e[XROW : XROW + BN, :],
                in0=bm_sl,
                scalar1=e_col[XROW : XROW + BN, 0:1],
                scalar2=None,
                op0=mybir.AluOpType.mult,
            )
            # Weight cast placed AFTER rhs_e in the DVE stream: it is 430 ns
            # of DVE time and must not delay the critical exp->rhs_e->y path
            # (the tile scheduler keeps per-engine program order here).
            sb_w = sb.tile([128, CINB_COLS], bf16)
            nc.vector.tensor_copy(out=sb_w[:], in_=cinb[:])

            # denom[b] = sum_n e ; y[h, b] = sum_n x * e  (contract over bn).
            ps_den = ps.tile([1, BL], f32, tag="den")
            nc.tensor.matmul(
                ps_den[:],
                ones_col[XROW : XROW + BN, :],
                rhs_e[XROW : XROW + BN, :],
                start=True,
                stop=True,
            )
            ps_y = ps.tile([128, 2 * BL], f32, tag="y")
            for kh in range(2):
                nc.tensor.matmul(
                    ps_y[:, kh * BL : (kh + 1) * BL],
                    sb_x[XROW : XROW + BN, kh * 128 : (kh + 1) * 128],
                    rhs_e[XROW : XROW + BN, :],
                    start=True,
                    stop=True,
                )
            recip = sb.tile([1, BL], bf16)
            with nc.allow_low_precision(reason="1/denom in bf16; tol 2e-2"):
                nc.vector.reciprocal(recip[:], ps_den[:])
            s_y = sb.tile([128, 2 * BL], bf16)
            nc.vector.tensor_copy(out=s_y[:], in_=ps_y[:])

            # v[c, b] = sum_h W[h, c] * y[h, b]   (c-half per group).
            ps_v = ps.tile([128, 2 * BL], f32, tag="v")
            for ch in range(2):
                for kh in range(2):
                    nc.tensor.matmul(
                        ps_v[:, ch * BL : (ch + 1) * BL],
                        sb_w[:, kh * C + ch * 128 : kh * C + (ch + 1) * 128],
                        s_y[:, kh * BL : (kh + 1) * BL],
                        start=(kh == 0),
                        stop=(kh == 1),
                    )

            # Broadcast 1/denom to all partitions with a K=1 matmul, placed
            # AFTER the v matmuls so its reciprocal wait never stalls them
            # (GPSIMD cannot read PSUM and DVE lanes cannot cross
            # partitions), then funnel v and 1/denom to SBUF on DVE so
            # every fill below needs at most one sync wait (HW limit).
            ps_r = ps.tile([128, BL], f32, tag="r")
            nc.tensor.matmul(ps_r[:], ones_row[:], recip[:], start=True, stop=True)
            s_v = sb.tile([128, 2 * BL], f32)
            nc.vector.tensor_copy(out=s_v[:], in_=ps_v[:])
            s_rr = sb.tile([128, BL], f32)
            nc.vector.tensor_copy(out=s_rr[:], in_=ps_r[:])

            # Normalize + ReLU + broadcast-fill one CH1-wide chunk per
            # (batch, c-half); each store DMA replicates its chunk over all
            # pixels via a stride-0 repeat dim in the source access
            # pattern.  Low c-halves fill on DVE, high c-halves on the
            # scalar engine (activation Relu with per-partition scale), so
            # the two engines fill concurrently.  All output DMAs trigger
            # from the scalar engine: its queue's descriptor generation
            # leaves the DMA data engines at line rate, whereas the sync
            # queue's generator steals ~35% of engine E79's bandwidth for
            # the duration of each DMA's generation window, stretching the
            # stream tail by several us.
            def dve_fill(dst, b, width):
                nc.vector.tensor_scalar(
                    out=dst,
                    in0=_rep_ap(s_v[:, b : b + 1], [[0, width]]),
                    scalar1=s_rr[:, b : b + 1],
                    scalar2=0.0,
                    op0=mybir.AluOpType.mult,
                    op1=mybir.AluOpType.max,
                )

            def act_fill(dst, b, width):
                nc.scalar.activation(
                    dst,
                    _rep_ap(s_v[:, BL + b : BL + b + 1], [[0, width]]),
                    mybir.ActivationFunctionType.Relu,
                    scale=s_rr[:, b : b + 1],
                )

            bc0 = sb.tile([128, 2 * CH1], f32, tag="bc0")
            bc1 = sb.tile([128, 2 * CH1], f32, tag="bc1")
            dve_fill(bc0[:, 0:CH1], 0, CH1)
            dve_fill(bc1[:, 0:CH1], 1, CH1)
            nc.scalar.dma_start(
                out=_rep_ap(out_d[0][0:128, :], [[CH1, NREP1], [1, CH1]]),
                in_=_rep_ap(bc0[:, 0:CH1], [[0, NREP1], [1, CH1]]),
            )
            act_fill(bc0[:, CH1 : 2 * CH1], 0, CH1)
            nc.scalar.dma_start(
                out=_rep_ap(out_d[0][128:256, :], [[CH1, NREP1], [1, CH1]]),
                in_=_rep_ap(bc0[:, CH1 : 2 * CH1], [[0, NREP1], [1, CH1]]),
            )
            nc.scalar.dma_start(
                out=_rep_ap(out_d[1][0:128, :], [[CH1, NREP1], [1, CH1]]),
                in_=_rep_ap(bc1[:, 0:CH1], [[0, NREP1], [1, CH1]]),
            )
            act_fill(bc1[:, CH1 : 2 * CH1], 1, CH1)
            nc.scalar.dma_start(
                out=_rep_ap(out_d[1][128:256, :], [[CH1, NREP1], [1, CH1]]),
                in_=_rep_ap(bc1[:, CH1 : 2 * CH1], [[0, NREP1], [1, CH1]]),
            )
    _fix_tail_drain(nc)
    return nc


def _fix_tail_drain(nc):
    """Walrus accepts very few sync waits per instruction, and Tile's
    kernel-tail drain waits on every semaphore. The whole dataflow funnels
    into the four output DMAs, all FIFO on the sync queue, so waiting on
    the LAST one's completion sem alone is sufficient. Strip the drain
    down to that wait."""
    import bass_rust

    out_sem = None
    for ins in nc.inst_map.values():
        if type(ins).__name__ == "InstDMACopy" and "out_set" in str(ins):
            si = ins.sync_info
            if si is not None and len(si.on_update) > 0:
                out_sem = si.on_update[0].ant_name
    assert out_sem is not None, "output DMA completion sem not found"
    for ins in nc.inst_map.values():
        si = ins.sync_info
        if type(ins).__name__ == "InstDrain" and si is not None and len(si.on_wait) > 1:
            keep = [w for w in si.on_wait if w.ant_name == out_sem]
            assert len(keep) == 1, (out_sem, [w.ant_name for w in si.on_wait])
            ins.sync_info = bass_rust.SyncInfo(
                on_wait=keep, on_update=list(si.on_update)
            )


def _get_nc():
    if "nc" not in _cache:
        _cache["nc"] = _build_nc()
    return _cache["nc"]


def _pack_cina(x_shard, nfh):
    cina = np.zeros((128, CINA_COLS), dtype=np.float32)
    cina[XROW : XROW + BN, COL_X : COL_X + HID] = x_shard.reshape(BN, HID)
    cina[XROW : XROW + BN, COL_NFH : COL_NFH + HID] = nfh[:, 0][None, :]
    for b in range(BL):
        cina[XROW + b * NODES : XROW + (b + 1) * NODES, COL_BM + b] = 1.0
    return cina


def _pack_cinb(w):
    cinb = np.zeros((128, CINB_COLS), dtype=np.float32)
    cinb[:, 0:C] = w[0:128, :]
    cinb[:, C : 2 * C] = w[128:256, :]
    return cinb


def _make_in_maps(input, node_fea_for_hidden, weight):
    x_full = np.asarray(input, dtype=np.float32)[0]  # (B, N, HID)
    nfh = np.asarray(node_fea_for_hidden, dtype=np.float32)
    w = np.asarray(weight, dtype=np.float32)
    cinb = _pack_cinb(w)
    return [
        {"cina": _pack_cina(x_full[i * BL : (i + 1) * BL], nfh), "cinb": cinb}
        for i in range(NCORES)
    ]


def _run(in_maps, trace=False, **kwargs):
    nc = _get_nc()
    return run_bass_kernel_spmd(nc, in_maps, list(range(NCORES)), trace=trace, **kwargs)


def kernel(input, res_feature, node_fea_for_res, node_fea_for_hidden, weight):
    in_maps = _make_in_maps(input, node_fea_for_hidden, weight)
    res = _run(in_maps)
    shards = [res.results[i]["out"] for i in range(NCORES)]  # each (BL, C, P)
    full = np.concatenate(shards, axis=0)  # (B, C, P)
    return full.reshape(B, C, H, W).astype(np.float32, copy=False)


# revision 18
# speedup vs baseline: 1.1588x; 1.1572x over previous
"""Trainium2 Bass kernel for nn_Graph_to_Featuremaps_savemem.

Reference computation:
    scores[b,p,n] = s_res[b,p] + s_hid[b,n];  attn = softmax_n(scores)
    out[b,c,p]    = relu(sum_n attn[b,p,n] * (x[b,n,:] @ W)[c])

Key simplification: softmax over n is shift-invariant, so the per-pixel
s_res[b,p] term (the only use of res_feature / node_fea_for_res) cancels:
    attn[b,p,n] = softmax_n(s_hid[b,n])   (independent of p)
    out[b,c,p]  = relu(sum_n a[b,n] * nv[b,n,c])  broadcast over all pixels.

So the kernel is a tiny softmax-weighted matmul per batch followed by a
151 MB broadcast-write of the (B, C) result over H*W pixels. Sharding:
data-parallel over batch, 2 batches per core across 8 cores; the small
params (node_fea_for_hidden, weight) are replicated.

The structure targets the DMA-store roofline (~26 GB/s x 16 DMA engines
per core): the 18.9 MB/core output stream goes on the wire as early as
possible and everything else hides beneath it.

  - The output broadcast is NOT materialized in SBUF.  Per (batch, c-half)
    only one (128, CH) chunk is filled (CH = 2304 pixels); the store DMA's
    *source* access pattern revisits it with a stride-0 repeat dim, so the
    DMA replicates it across all 9216 pixels.  This removes the baseline's
    2x 9.4 MB DVE broadcast fills (23.8 us) from the critical path.
    CH is chosen so descriptors are 9.2 KB: at 4.6 KB the descriptor
    generator falls ~6% short of the 16-engine line rate and the last
    engine in the round-robin accumulates an 8 us straggle.
  - All DMAs ride the sync-engine queue (its trigger is ~2x faster than
    the scalar engine's, and queue FIFO order lets the tail drain wait on
    the final DMA's semaphore alone).
  - s_hid = x . nfh is a DVE multiply + free-dim reduce against a
    host-packed nfh replica -- no PE transposes anywhere.
  - softmax normalization is deferred: y = x^T (blockmask * exp(s)) and
    v = W^T y use unnormalized weights; 1/denom and the ReLU are fused
    into the chunk fills (DVE tensor_scalar mult+max for the low c-half,
    scalar-engine activation Relu-with-scale for the high c-half, running
    concurrently).  v and 1/denom are funneled through GPSIMD copies so
    every fill carries a single sync wait (HW limit).
  - matmuls run in bf16 (O(1) gaussian data; tolerance 2e-2, measured
    error ~3e-3).
"""

import numpy as np

import concourse.bass as bass
import concourse.mybir as mybir
import concourse.tile as tile
from concourse.bass_utils import run_bass_kernel_spmd

B, NODES, HID, C, H, W = 16, 7, 256, 256, 96, 96
P = H * W                # 9216 pixels
NCORES = 8
BL = B // NCORES         # 2 local batches per core
BN = BL * NODES          # 14 (b,n) rows
CH1 = 4608               # chunk width of the FIRST store DMA (pixels)
NREP1 = P // CH1         # its stride-0 repeat count

# cin_a (small, loaded first; only rows 32:46 are transferred):
#   cols 0:256 x[(b n), h]; 256:512 nfh replicated per row; 512:514 blockmask
XROW = 32                # base partition for the 14 (b,n) rows (PE: 0/32/64)
COL_X = 0
COL_NFH = 256
COL_BM = 512
CINA_COLS = 514
# cin_b: W packed [k, kh*256 + c] (k = h % 128, kh = h // 128)
CINB_COLS = 2 * C

_cache: dict = {}


def _rep_ap(ap, dims):
    """Return a copy of `ap` with its non-partition dims replaced by `dims`
    (list of [stride, count]); used to build stride-0 broadcast patterns."""
    a = ap.copy()
    a.ap = mybir.VecI64Pair([list(a.ap[0])] + [list(d) for d in dims])
    return a


def _build_nc():
    nc = bass.Bass()
    f32 = mybir.dt.float32
    bf16 = mybir.dt.bfloat16
    cina_d = nc.declare_dram_parameter("cina", [128, CINA_COLS], f32, isOutput=False)
    cinb_d = nc.declare_dram_parameter("cinb", [128, CINB_COLS], f32, isOutput=False)
    out_d = nc.declare_dram_parameter("out", [BL, C, P], f32, isOutput=True)

    with tile.TileContext(nc) as tc:
        with (
            tc.tile_pool(name="sb", bufs=1) as sb,
            tc.tile_pool(name="ps", bufs=1, space=bass.MemorySpace.PSUM) as ps,
        ):
            cina = sb.tile([128, CINA_COLS], f32)
            cinb = sb.tile([128, CINB_COLS], f32)
            # The two input loads trigger concurrently from different
            # engines, so the weight lands before the exp(s) chain needs the
            # DVE and its cast never blocks the critical path.
            nc.sync.dma_start(
                out=cina[XROW : XROW + BN, :], in_=cina_d[XROW : XROW + BN, :]
            )
            nc.scalar.dma_start(out=cinb[:], in_=cinb_d[:])

            x_sl = cina[XROW : XROW + BN, COL_X : COL_X + HID]
            nfh_sl = cina[XROW : XROW + BN, COL_NFH : COL_NFH + HID]
            bm_sl = cina[XROW : XROW + BN, COL_BM : COL_BM + BL]

            # DVE-produced matmul operands (single-producer rule for PE).
            ones_col = sb.tile([128, 1], bf16)
            nc.vector.memset(ones_col[:], 1.0)
            ones_row = sb.tile([1, 128], bf16)
            nc.vector.memset(ones_row[:], 1.0)

            # s[(b n)] = sum_h x * nfh  (multiply + free-dim reduce).
            tt_scratch = sb.tile([128, HID], f32)
            s_col = sb.tile([128, 1], f32)
            nc.vector.tensor_tensor(
                out=tt_scratch[XROW : XROW + BN, :],
                in0=x_sl,
                in1=nfh_sl,
                op=mybir.AluOpType.mult,
            )
            nc.vector.tensor_reduce(
                out=s_col[XROW : XROW + BN, :],
                in_=tt_scratch[XROW : XROW + BN, :],
                axis=mybir.AxisListType.X,
                op=mybir.AluOpType.add,
            )
            sb_x = sb.tile([128, HID], bf16)
            nc.vector.tensor_copy(out=sb_x[XROW : XROW + BN, :], in_=x_sl)

            # e = exp(s) on the scalar engine (normalization deferred).
            e_col = sb.tile([128, 1], f32)
            nc.scalar.activation(
                e_col[XROW : XROW + BN, :],
                s_col[XROW : XROW + BN, :],
                mybir.ActivationFunctionType.Exp,
            )
            # rhs_e[(b n), b'] = blockmask * e  (unnormalized per-batch attn).
            rhs_e = sb.tile([128, BL], bf16)
            nc.vector.tensor_scalar(
                out=rhs_e[XROW : XROW + BN, :],
                in0=bm_sl,
                scalar1=e_col[XROW : XROW + BN, 0:1],
                scalar2=None,
                op0=mybir.AluOpType.mult,
            )
            # Weight cast placed AFTER rhs_e in the DVE stream: it is 430 ns
            # of DVE time and must not delay the critical exp->rhs_e->y path
            # (the tile scheduler keeps per-engine program order here).
            sb_w = sb.tile([128, CINB_COLS], bf16)
            nc.vector.tensor_copy(out=sb_w[:], in_=cinb[:])

            # denom[b] = sum_n e ; y[h, b] = sum_n x * e  (contract over bn).
            ps_den = ps.tile([1, BL], f32, tag="den")
            nc.tensor.matmul(
                ps_den[:],
                ones_col[XROW : XROW + BN, :],
                rhs_e[XROW : XROW + BN, :],
                start=True,
                stop=True,
            )
            ps_y = ps.tile([128, 2 * BL], f32, tag="y")
            for kh in range(2):
                nc.tensor.matmul(
                    ps_y[:, kh * BL : (kh + 1) * BL],
                    sb_x[XROW : XROW + BN, kh * 128 : (kh + 1) * 128],
                    rhs_e[XROW : XROW + BN, :],
                    start=True,
                    stop=True,
                )
            recip = sb.tile([1, BL], bf16)
            with nc.allow_low_precision(reason="1/denom in bf16; tol 2e-2"):
                nc.vector.reciprocal(recip[:], ps_den[:])
            s_y = sb.tile([128, 2 * BL], bf16)
            nc.vector.tensor_copy(out=s_y[:], in_=ps_y[:])

            # v[c, b] = sum_h W[h, c] * y[h, b]   (c-half per group).
            ps_v = ps.tile([128, 2 * BL], f32, tag="v")
            for ch in range(2):
                for kh in range(2):
                    nc.tensor.matmul(
                        ps_v[:, ch * BL : (ch + 1) * BL],
                        sb_w[:, kh * C + ch * 128 : kh * C + (ch + 1) * 128],
                        s_y[:, kh * BL : (kh + 1) * BL],
                        start=(kh == 0),
                        stop=(kh == 1),
                    )

            # Broadcast 1/denom to all partitions with a K=1 matmul, placed
            # AFTER the v matmuls so its reciprocal wait never stalls them
            # (GPSIMD cannot read PSUM and DVE lanes cannot cross
            # partitions), then funnel v and 1/denom to SBUF on DVE so
            # every fill below needs at most one sync wait (HW limit).
            ps_r = ps.tile([128, BL], f32, tag="r")
            nc.tensor.matmul(ps_r[:], ones_row[:], recip[:], start=True, stop=True)
            s_v = sb.tile([128, 2 * BL], f32)
            nc.vector.tensor_copy(out=s_v[:], in_=ps_v[:])
            s_rr = sb.tile([128, BL], f32)
            nc.vector.tensor_copy(out=s_rr[:], in_=ps_r[:])

            # Normalize + ReLU + materialize the broadcast, then store with
            # PLAIN (non-replicating) DMAs.  Stride-0 replicating source
            # patterns perturb DMA engine E79 (which also hosts descriptor
            # generation) and stretch the stream tail 4-10 us, so the full
            # (128, 2, P) image per batch is materialized, exactly like the
            # clean baseline stream.  The stream still starts early because
            # the first store covers only pixels 0:CH1 of batch0/c-low
            # (ready after a 2.6 us fill); DVE fills the low c-halves while
            # the scalar engine (activation Relu with per-partition scale)
            # concurrently fills the high c-halves.  All output DMAs
            # trigger from the otherwise-idle sync engine, in fill-
            # completion order, and each trigger carries one sync wait.
            def dve_fill(dst, b, width):
                nc.vector.tensor_scalar(
                    out=dst,
                    in0=_rep_ap(s_v[:, b : b + 1], [[0, width]]),
                    scalar1=s_rr[:, b : b + 1],
                    scalar2=0.0,
                    op0=mybir.AluOpType.mult,
                    op1=mybir.AluOpType.max,
                )

            def act_fill(dst, b, width):
                nc.scalar.activation(
                    dst,
                    _rep_ap(s_v[:, BL + b : BL + b + 1], [[0, width]]),
                    mybir.ActivationFunctionType.Relu,
                    scale=s_rr[:, b : b + 1],
                )

            bc0 = sb.tile([128, 2 * P], f32, tag="bc0")
            bc1 = sb.tile([128, 2 * P], f32, tag="bc1")
            # DVE: b0/c-low in two pieces (first store leaves early), then
            # b1/c-low.  Scalar: b0/c-high, b1/c-high (concurrently).
            dve_fill(bc0[:, 0:CH1], 0, CH1)
            nc.sync.dma_start(out=out_d[0][0:128, 0:CH1], in_=bc0[:, 0:CH1])
            dve_fill(bc0[:, CH1:P], 0, P - CH1)
            nc.sync.dma_start(out=out_d[0][0:128, CH1:P], in_=bc0[:, CH1:P])
            act_fill(bc0[:, P : 2 * P], 0, P)
            nc.sync.dma_start(out=out_d[0][128:256, :], in_=bc0[:, P : 2 * P])
            dve_fill(bc1[:, 0:P], 1, P)
            nc.sync.dma_start(out=out_d[1][0:128, :], in_=bc1[:, 0:P])
            act_fill(bc1[:, P : 2 * P], 1, P)
            nc.sync.dma_start(out=out_d[1][128:256, :], in_=bc1[:, P : 2 * P])
    _fix_tail_drain(nc)
    return nc


def _fix_tail_drain(nc):
    """Walrus accepts very few sync waits per instruction, and Tile's
    kernel-tail drain waits on every semaphore. The whole dataflow funnels
    into the four output DMAs, all FIFO on the sync queue, so waiting on
    the LAST one's completion sem alone is sufficient. Strip the drain
    down to that wait."""
    import bass_rust

    out_sem = None
    for ins in nc.inst_map.values():
        if type(ins).__name__ == "InstDMACopy" and "out_set" in str(ins):
            si = ins.sync_info
            if si is not None and len(si.on_update) > 0:
                out_sem = si.on_update[0].ant_name
    assert out_sem is not None, "output DMA completion sem not found"
    for ins in nc.inst_map.values():
        si = ins.sync_info
        if type(ins).__name__ == "InstDrain" and si is not None and len(si.on_wait) > 1:
            keep = [w for w in si.on_wait if w.ant_name == out_sem]
            assert len(keep) == 1, (out_sem, [w.ant_name for w in si.on_wait])
            ins.sync_info = bass_rust.SyncInfo(
                on_wait=keep, on_update=list(si.on_update)
            )


def _get_nc():
    if "nc" not in _cache:
        _cache["nc"] = _build_nc()
    return _cache["nc"]


def _pack_cina(x_shard, nfh):
    cina = np.zeros((128, CINA_COLS), dtype=np.float32)
    cina[XROW : XROW + BN, COL_X : COL_X + HID] = x_shard.reshape(BN, HID)
    cina[XROW : XROW + BN, COL_NFH : COL_NFH + HID] = nfh[:, 0][None, :]
    for b in range(BL):
        cina[XROW + b * NODES : XROW + (b + 1) * NODES, COL_BM + b] = 1.0
    return cina


def _pack_cinb(w):
    cinb = np.zeros((128, CINB_COLS), dtype=np.float32)
    cinb[:, 0:C] = w[0:128, :]
    cinb[:, C : 2 * C] = w[128:256, :]
    return cinb


def _make_in_maps(input, node_fea_for_hidden, weight):
    x_full = np.asarray(input, dtype=np.float32)[0]  # (B, N, HID)
    nfh = np.asarray(node_fea_for_hidden, dtype=np.float32)
    w = np.asarray(weight, dtype=np.float32)
    cinb = _pack_cinb(w)
    return [
        {"cina": _pack_cina(x_full[i * BL : (i + 1) * BL], nfh), "cinb": cinb}
        for i in range(NCORES)
    ]


def _run(in_maps, trace=False, **kwargs):
    nc = _get_nc()
    return run_bass_kernel_spmd(nc, in_maps, list(range(NCORES)), trace=trace, **kwargs)


def kernel(input, res_feature, node_fea_for_res, node_fea_for_hidden, weight):
    in_maps = _make_in_maps(input, node_fea_for_hidden, weight)
    res = _run(in_maps)
    shards = [res.results[i]["out"] for i in range(NCORES)]  # each (BL, C, P)
    full = np.concatenate(shards, axis=0)  # (B, C, P)
    return full.reshape(B, C, H, W).astype(np.float32, copy=False)
